# revision 10
# baseline (speedup 1.0000x reference)
"""Trainium2 Bass kernel for nn_AAFM (sparse attention with distance decay).

Math (per batch b):
    q = query @ Wq.T + bq ; k = key @ Wk.T + bk ; v = value @ Wv.T + bv
    exp_A = exp(-alpha*log2(N)*dist)            [n_q, n_k]
    num   = exp_A @ (exp(k) * v)                [n_q, d]
    den   = exp_A @ exp(k) + 1e-8               [n_q, d]
    out   = sigmoid(q) * num / den

Transforms used here:
  - bk cancels exactly in num/den (columnwise positive factor exp(bk)), so it
    is dropped (the 1e-8 placement differs by a negligible ~1e-10 relative).
  - bv folds OUT of the matmul: exp_A @ (ek*(v+bv)) = exp_A @ (ek*v) + den*bv,
    so out = sig(q) * (num'/den + bv). No bias matmuls on the PE at all.
  - sigmoid(q) = 0.5*(tanh(q/2)+1): tanh shares the ACT "exp_and_others"
    table set with exp, avoiding per-batch ~2.7us table reloads. The 0.5 is
    folded into Wv/bv on the host: out = (tanh(q/2)+1) * (num'/den + bv').
  - bq is applied by a vector STT (0.5*qp + 0.5*bq broadcast) before tanh.
  - All device DMAs are natural-layout: the host pre-transposes dist to
    [b, k, q] and q/k/v to [b, d, n] and pre-transposes the weights, so the
    TensorEngine contraction dim is always the SBUF partition dim.
  - q/k/v DMAs cast f32->bf16 in flight (gpsimd-initiated DGE cast).
  - Output is stored/DMA'd as bf16 and upcast on the host.
  - Data-parallel over batch: 32 batches -> 8 cores x 4, no collectives.
  - fp8 was evaluated and rejected: the numerator suffers sign cancellation
    (weighted mean of v ~ 0) which amplifies e4m3 quantization to ~5% rel err.
"""

import sys

for _p in ("/opt/trn_rl_repo",):
    if _p not in sys.path:
        sys.path.append(_p)

import numpy as np

import concourse.bass as bass
import concourse.tile as tile
from concourse import bacc, mybir
from concourse.bass_utils import run_bass_kernel_spmd

N_CORES = 8
B = 32
N = 1024  # graph size
D = 256  # d_model
BPC = B // N_CORES  # batches per core
KT = N // 128  # 8 row tiles of 128
F32 = mybir.dt.float32
BF16 = mybir.dt.bfloat16
Alu = mybir.AluOpType
Act = mybir.ActivationFunctionType


def build_graph(c_coef: float):
    """Build the SPMD single-core graph (same on all 8 cores)."""
    nc = bacc.Bacc(
        "TRN2", target_bir_lowering=False, debug=False, num_devices=N_CORES
    )

    distT = nc.declare_dram_parameter("distT", [BPC, N, N], F32, isOutput=False)
    qT = nc.declare_dram_parameter("qT", [BPC, D, N], F32, isOutput=False)
    kT = nc.declare_dram_parameter("kT", [BPC, D, N], F32, isOutput=False)
    vT = nc.declare_dram_parameter("vT", [BPC, D, N], F32, isOutput=False)
    WqT = nc.declare_dram_parameter("WqT", [D, D], F32, isOutput=False)
    WkT = nc.declare_dram_parameter("WkT", [D, D], F32, isOutput=False)
    WvT = nc.declare_dram_parameter("WvT", [D, D], F32, isOutput=False)
    bqh_d = nc.declare_dram_parameter("bqh", [128, D], F32, isOutput=False)
    bvh_d = nc.declare_dram_parameter("bvh", [128, D], F32, isOutput=False)
    out_d = nc.declare_dram_parameter("out", [BPC, N, D], BF16, isOutput=True)

    with tile.TileContext(nc) as tc:
        with (
            tc.tile_pool(name="const", bufs=1) as const_p,
            tc.tile_pool(name="qkv", bufs=2) as qkv_p,
            tc.tile_pool(name="dist", bufs=6) as dist_p,
            tc.tile_pool(name="expA", bufs=2) as expA_p,
            tc.tile_pool(name="ekv", bufs=2) as ekv_p,
            tc.tile_pool(name="tanh", bufs=2) as tanh_p,
            tc.tile_pool(name="eps", bufs=3) as eps_p,
            tc.tile_pool(name="outst", bufs=2) as out_p,
            tc.tile_pool(name="ppsum", bufs=2, space="PSUM") as ppsum,
            tc.tile_pool(name="qpsum", bufs=2, space="PSUM") as qpsum,
            tc.tile_pool(name="mpsum", bufs=2, space="PSUM") as mpsum,
        ):
            # ---- constants (once) ----
            w_tiles = {}
            for nm, wd in (("wk", WkT), ("wv", WvT), ("wq", WqT)):
                wt = const_p.tile([128, 2, D], BF16, tag=nm)
                nc.gpsimd.dma_start(
                    wt[:], wd[:].rearrange("(j p) e -> p j e", p=128)
                )
                w_tiles[nm] = wt
            bqh_t = const_p.tile([128, D], F32, tag="bqh")
            nc.gpsimd.dma_start(bqh_t[:], bqh_d[:])
            bvh_t = const_p.tile([128, D], F32, tag="bvh")
            nc.gpsimd.dma_start(bvh_t[:], bvh_d[:])

            # PE warm-up: keep the HAM activity monitor busy while the first
            # input DMAs land, so real matmuls start at 2.4 GHz.
            warm_sb = const_p.tile([128, 512], BF16, tag="warm")
            nc.vector.memset(warm_sb[:], 0.0)
            warm_ps = mpsum.tile([128, 512], F32, tag="mm")
            for _ in range(32):
                nc.tensor.matmul(
                    warm_ps[:], warm_sb[:, 0:128], warm_sb[:], start=True, stop=True
                )

            def phase_load(bi):
                # qkv casts first so proj(b) can start as soon as possible;
                # dist quarters after (consumed later, by expA ACTs).
                xt = {}
                for nm, xd in (("kT", kT), ("vT", vT), ("qT", qT)):
                    t = qkv_p.tile([128, 2, N], BF16, tag=nm)
                    for dt in range(2):
                        nc.gpsimd.dma_start(
                            t[:, dt, :], xd[bi, dt * 128 : (dt + 1) * 128, :]
                        )
                    xt[nm] = t
                dists = []
                for c in range(4):
                    dt_t = dist_p.tile([128, 2, N], F32, tag="dist")
                    nc.sync.dma_start(
                        dt_t[:],
                        distT[bi, c * 256 : (c + 1) * 256, :].rearrange(
                            "(j p) q -> p j q", p=128
                        ),
                    )
                    dists.append(dt_t)
                return xt, dists

            def phase_proj(xt, dists, first=False):
                expA = expA_p.tile([128, KT, N], BF16, tag="expA")
                ekv = ekv_p.tile([128, KT, 2 * D], BF16, tag="ekv")
                tanh_t = tanh_p.tile([128, KT, D], BF16, tag="tanh")

                def expa_chunk(c):
                    nc.scalar.activation(
                        expA[:, c * 2 : (c + 1) * 2, :],
                        dists[c][:],
                        Act.Exp,
                        scale=-c_coef,
                    )

                if first:
                    # Prologue: dist arrives before the PE finishes warmup, so
                    # front-load the first expA chunks to unblock main(0).
                    expa_chunk(0)
                    expa_chunk(1)
                for t0 in range(KT):
                    cols = slice(t0 * 128, (t0 + 1) * 128)
                    kv = ppsum.tile([128, 2, D], F32, tag="kv")
                    for dt in range(2):
                        nc.tensor.matmul(
                            kv[:, 0, :],
                            xt["kT"][:, dt, cols],
                            w_tiles["wk"][:, dt, :],
                            start=(dt == 0),
                            stop=(dt == 1),
                        )
                    for dt in range(2):
                        nc.tensor.matmul(
                            kv[:, 1, :],
                            xt["vT"][:, dt, cols],
                            w_tiles["wv"][:, dt, :],
                            start=(dt == 0),
                            stop=(dt == 1),
                        )
                    nc.scalar.activation(
                        ekv[:, t0, D : 2 * D], kv[:, 0, :], Act.Exp
                    )
                    nc.vector.scalar_tensor_tensor(
                        ekv[:, t0, 0:D],
                        kv[:, 1, :],
                        1.0,
                        ekv[:, t0, D : 2 * D],
                        Alu.mult,
                        Alu.mult,
                    )

                    qp = qpsum.tile([128, D], F32, tag="qp")
                    for dt in range(2):
                        nc.tensor.matmul(
                            qp[:],
                            xt["qT"][:, dt, cols],
                            w_tiles["wq"][:, dt, :],
                            start=(dt == 0),
                            stop=(dt == 1),
                        )
                    qb = eps_p.tile([128, D], F32, tag="qb")
                    nc.vector.scalar_tensor_tensor(
                        qb[:], qp[:], 0.5, bqh_t[:], Alu.mult, Alu.add
                    )
                    nc.scalar.activation(
                        tanh_t[:, t0, :], qb[:], Act.Tanh
                    )
                # expA exps go LAST on the scalar queue: they are bulk work
                # (1.8us per chunk) needed only by main(b+1), while the small
                # ek/tanh ACTs above gate the PE's kv-psum recycling.
                for c in range(0 if not first else 2, 4):
                    expa_chunk(c)
                return expA, ekv, tanh_t

            def phase_main(bi, expA, ekv, tanh_t):
                out_t = out_p.tile([128, KT, D], BF16, tag="outst")
                for qi in range(KT):
                    mm = mpsum.tile([128, 512], F32, tag="mm")
                    for t in range(KT):
                        nc.tensor.matmul(
                            mm[:],
                            expA[:, t, qi * 128 : (qi + 1) * 128],
                            ekv[:, t, :],
                            start=(t == 0),
                            stop=(t == KT - 1),
                        )
                    r = eps_p.tile([128, D], F32, tag="r")
                    nc.vector.reciprocal_approx_fast(r[:], mm[:, D : 2 * D])
                    m = eps_p.tile([128, D], F32, tag="m")
                    nc.vector.tensor_mul(m[:], mm[:, 0:D], r[:])
                    a = eps_p.tile([128, D], F32, tag="a")
                    nc.gpsimd.tensor_add(a[:], m[:], bvh_t[:])
                    nc.vector.scalar_tensor_tensor(
                        out_t[:, qi, :], tanh_t[:, qi, :], 1.0, a[:],
                        Alu.add, Alu.mult,
                    )
                for h in range(2):
                    nc.gpsimd.dma_start(
                        out_d[bi, h * 512 : (h + 1) * 512, :].rearrange(
                            "(j p) e -> p j e", p=128
                        ),
                        out_t[:, h * 4 : (h + 1) * 4, :],
                    )

            # software pipeline: DMAs for b+1 are issued before main(b); the
            # PE runs main(b) (data already resident) and only then proj(b+1),
            # so it never head-of-line blocks on next-batch loads.
            staged = phase_proj(*phase_load(0), first=True)
            for bi in range(BPC):
                ld = phase_load(bi + 1) if bi + 1 < BPC else None
                phase_main(bi, *staged)
                staged = phase_proj(*ld) if ld is not None else None

    nc.compile()
    return nc


def prepare_in_maps(inputs: dict):
    query = np.asarray(inputs["query"], dtype=np.float32)
    key_ = np.asarray(inputs["key_"], dtype=np.float32)
    value = np.asarray(inputs["value"], dtype=np.float32)
    dist = np.asarray(inputs["dist"], dtype=np.float32)
    Wq = np.asarray(inputs["Wq"], dtype=np.float32)
    Wk = np.asarray(inputs["Wk"], dtype=np.float32)
    Wv = np.asarray(inputs["Wv"], dtype=np.float32)
    bq = np.asarray(inputs["bq"], dtype=np.float32)
    bv = np.asarray(inputs["bv"], dtype=np.float32)
    alpha_raw = np.asarray(inputs["alpha_raw"], dtype=np.float64)

    alpha = float(np.logaddexp(0.0, alpha_raw)) + 1e-6  # softplus + eps
    c_coef = float(alpha * np.log2(float(N)))

    distT = np.ascontiguousarray(dist.transpose(0, 2, 1))
    qT = np.ascontiguousarray(query.transpose(0, 2, 1))
    kT = np.ascontiguousarray(key_.transpose(0, 2, 1))
    vT = np.ascontiguousarray(value.transpose(0, 2, 1))
    WqT = np.ascontiguousarray(Wq.T)
    WkT = np.ascontiguousarray(Wk.T)
    WvT = np.ascontiguousarray(Wv.T) * 0.5  # fold sigmoid's 0.5
    # broadcast biases to 128 partitions; 0.5 folds sigmoid's scale
    bqh = np.ascontiguousarray(
        np.broadcast_to(0.5 * bq.reshape(1, D), (128, D)).astype(np.float32)
    )
    bvh = np.ascontiguousarray(
        np.broadcast_to(0.5 * bv.reshape(1, D), (128, D)).astype(np.float32)
    )

    in_maps = []
    for i in range(N_CORES):
        s = slice(i * BPC, (i + 1) * BPC)
        in_maps.append(
            {
                "distT": distT[s],
                "qT": qT[s],
                "kT": kT[s],
                "vT": vT[s],
                "WqT": WqT,
                "WkT": WkT,
                "WvT": WvT,
                "bqh": bqh,
                "bvh": bvh,
            }
        )
    return in_maps, c_coef


def run_sharded(inputs: dict, trace: bool = False):
    """Returns (full_output [32,1024,256] f32, BassKernelResults)."""
    in_maps, c_coef = prepare_in_maps(inputs)
    nc = build_graph(c_coef)
    res = run_bass_kernel_spmd(
        nc, in_maps, core_ids=list(range(N_CORES)), trace=trace
    )
    out = np.concatenate(
        [np.asarray(res.results[i]["out"]) for i in range(N_CORES)], axis=0
    ).astype(np.float32)
    return out, res


def kernel(**inputs) -> np.ndarray:
    out, _ = run_sharded(inputs, trace=False)
    return out


# revision 18
# speedup vs baseline: 1.0591x; 1.0591x over previous
"""Trainium2 Bass kernel for nn_AAFM (sparse attention with distance decay).

Math (per batch b):
    q = query @ Wq.T + bq ; k = key @ Wk.T + bk ; v = value @ Wv.T + bv
    exp_A = exp(-alpha*log2(N)*dist)            [n_q, n_k]
    num   = exp_A @ (exp(k) * v)                [n_q, d]
    den   = exp_A @ exp(k) + 1e-8               [n_q, d]
    out   = sigmoid(q) * num / den

Transforms used here:
  - bk cancels exactly in num/den (columnwise positive factor exp(bk)), so it
    is dropped (the 1e-8 placement differs by a negligible ~1e-10 relative).
  - bv folds OUT of the matmul: exp_A @ (ek*(v+bv)) = exp_A @ (ek*v) + den*bv,
    so out = sig(q) * (num'/den + bv). No bias matmuls on the PE at all.
  - sigmoid(q) = 0.5*(tanh(q/2)+1): tanh shares the ACT "exp_and_others"
    table set with exp, avoiding per-batch ~2.7us table reloads. The 0.5 is
    folded into Wv/bv on the host: out = (tanh(q/2)+1) * (num'/den + bv').
  - bq is applied by a vector STT (0.5*qp + 0.5*bq broadcast) before tanh.
  - All device DMAs are natural-layout: the host pre-transposes dist to
    [b, k, q] and q/k/v to [b, d, n] and pre-transposes the weights, so the
    TensorEngine contraction dim is always the SBUF partition dim.
  - q/k/v DMAs cast f32->bf16 in flight (gpsimd-initiated DGE cast).
  - Output is stored/DMA'd as bf16 and upcast on the host.
  - Data-parallel over batch: 32 batches -> 8 cores x 4, no collectives.
  - fp8 was evaluated and rejected: the numerator suffers sign cancellation
    (weighted mean of v ~ 0) which amplifies e4m3 quantization to ~5% rel err.
"""

import sys

for _p in ("/opt/trn_rl_repo",):
    if _p not in sys.path:
        sys.path.append(_p)

import numpy as np

import concourse.bass as bass
import concourse.tile as tile
from concourse import bacc, mybir
from concourse.bass_utils import run_bass_kernel_spmd

N_CORES = 8
B = 32
N = 1024  # graph size
D = 256  # d_model
BPC = B // N_CORES  # batches per core
KT = N // 128  # 8 row tiles of 128
F32 = mybir.dt.float32
BF16 = mybir.dt.bfloat16
Alu = mybir.AluOpType
Act = mybir.ActivationFunctionType


def build_graph(c_coef: float):
    """Build the SPMD single-core graph (same on all 8 cores)."""
    nc = bacc.Bacc(
        "TRN2", target_bir_lowering=False, debug=False, num_devices=N_CORES
    )

    distT = nc.declare_dram_parameter("distT", [BPC, N, N], F32, isOutput=False)
    qT = nc.declare_dram_parameter("qT", [BPC, D, N], F32, isOutput=False)
    kT = nc.declare_dram_parameter("kT", [BPC, D, N], F32, isOutput=False)
    vT = nc.declare_dram_parameter("vT", [BPC, D, N], F32, isOutput=False)
    WqT = nc.declare_dram_parameter("WqT", [D, D], F32, isOutput=False)
    WkT = nc.declare_dram_parameter("WkT", [D, D], F32, isOutput=False)
    WvT = nc.declare_dram_parameter("WvT", [D, D], F32, isOutput=False)
    bqh_d = nc.declare_dram_parameter("bqh", [128, D], F32, isOutput=False)
    bvh_d = nc.declare_dram_parameter("bvh", [128, D], F32, isOutput=False)
    out_d = nc.declare_dram_parameter("out", [BPC, N, D], BF16, isOutput=True)

    with tile.TileContext(nc) as tc:
        with (
            tc.tile_pool(name="const", bufs=1) as const_p,
            tc.tile_pool(name="qkv", bufs=2) as qkv_p,
            tc.tile_pool(name="dist", bufs=6) as dist_p,
            tc.tile_pool(name="expA", bufs=2) as expA_p,
            tc.tile_pool(name="ekv", bufs=2) as ekv_p,
            tc.tile_pool(name="tanh", bufs=2) as tanh_p,
            tc.tile_pool(name="eps", bufs=3) as eps_p,
            tc.tile_pool(name="outst", bufs=2) as out_p,
            tc.tile_pool(name="ppsum", bufs=4, space="PSUM") as ppsum,
            tc.tile_pool(name="qpsum", bufs=2, space="PSUM") as qpsum,
            tc.tile_pool(name="mpsum", bufs=2, space="PSUM") as mpsum,
        ):
            # ---- constants (once) ----
            w_tiles = {}
            for nm, wd in (("wk", WkT), ("wv", WvT), ("wq", WqT)):
                wt = const_p.tile([128, 2, D], BF16, tag=nm)
                nc.gpsimd.dma_start(
                    wt[:], wd[:].rearrange("(j p) e -> p j e", p=128)
                )
                w_tiles[nm] = wt
            bqh_t = const_p.tile([128, D], F32, tag="bqh")
            nc.gpsimd.dma_start(bqh_t[:], bqh_d[:])
            bvh_t = const_p.tile([128, D], F32, tag="bvh")
            nc.gpsimd.dma_start(bvh_t[:], bvh_d[:])

            # PE warm-up: keep the HAM activity monitor busy while the first
            # input DMAs land, so real matmuls start at 2.4 GHz.
            warm_sb = const_p.tile([128, 512], BF16, tag="warm")
            nc.vector.memset(warm_sb[:], 0.0)
            warm_ps = mpsum.tile([128, 512], F32, tag="mm")
            for _ in range(20):
                nc.tensor.matmul(
                    warm_ps[:], warm_sb[:, 0:128], warm_sb[:], start=True, stop=True
                )

            def phase_load(bi, first=False):
                # qkv casts first so proj(b) can start as soon as possible;
                # dist quarters after (consumed later, by expA ACTs).
                # For batch 0 everything goes on the gpsimd DGE ring so the
                # hardware drains it strictly in this order (qkv before dist);
                # round-robin with the sync ring would starve the qkv casts.
                dist_eng = nc.gpsimd if first else nc.sync
                xt = {}
                for nm, xd in (("kT", kT), ("vT", vT), ("qT", qT)):
                    t = qkv_p.tile([128, 2, N], BF16, tag=nm)
                    for dt in range(2):
                        nc.gpsimd.dma_start(
                            t[:, dt, :], xd[bi, dt * 128 : (dt + 1) * 128, :]
                        )
                    xt[nm] = t
                dists = []
                for c in range(4):
                    dt_t = dist_p.tile([128, 2, N], F32, tag="dist")
                    dist_eng.dma_start(
                        dt_t[:],
                        distT[bi, c * 256 : (c + 1) * 256, :].rearrange(
                            "(j p) q -> p j q", p=128
                        ),
                    )
                    dists.append(dt_t)
                return xt, dists

            def phase_proj(xt, dists):
                expA = expA_p.tile([128, KT, N], BF16, tag="expA")
                ekv = ekv_p.tile([128, KT, 2 * D], BF16, tag="ekv")
                tanh_t = tanh_p.tile([128, KT, D], BF16, tag="tanh")

                def expa_chunk(c):
                    nc.scalar.activation(
                        expA[:, c * 2 : (c + 1) * 2, :],
                        dists[c][:],
                        Act.Exp,
                        scale=-c_coef,
                    )

                qb_all = tanh_p.tile([128, KT, D], F32, tag="qball")
                for t0 in range(KT):
                    cols = slice(t0 * 128, (t0 + 1) * 128)
                    kv = ppsum.tile([128, 2, D], F32, tag="kv")
                    for dt in range(2):
                        nc.tensor.matmul(
                            kv[:, 0, :],
                            xt["kT"][:, dt, cols],
                            w_tiles["wk"][:, dt, :],
                            start=(dt == 0),
                            stop=(dt == 1),
                        )
                    for dt in range(2):
                        nc.tensor.matmul(
                            kv[:, 1, :],
                            xt["vT"][:, dt, cols],
                            w_tiles["wv"][:, dt, :],
                            start=(dt == 0),
                            stop=(dt == 1),
                        )
                    nc.scalar.activation(
                        ekv[:, t0, D : 2 * D], kv[:, 0, :], Act.Exp
                    )
                    nc.vector.scalar_tensor_tensor(
                        ekv[:, t0, 0:D],
                        kv[:, 1, :],
                        1.0,
                        ekv[:, t0, D : 2 * D],
                        Alu.mult,
                        Alu.mult,
                    )

                    qp = qpsum.tile([128, D], F32, tag="qp")
                    for dt in range(2):
                        nc.tensor.matmul(
                            qp[:],
                            xt["qT"][:, dt, cols],
                            w_tiles["wq"][:, dt, :],
                            start=(dt == 0),
                            stop=(dt == 1),
                        )
                    nc.vector.scalar_tensor_tensor(
                        qb_all[:, t0, :], qp[:], 0.5, bqh_t[:], Alu.mult, Alu.add
                    )
                # Scalar queue order: the 8 small ek ACTs above gate the PE's
                # kv-psum recycling, so they go first; the bulk expA chunks
                # (needed by main(b+1)) and the batched tanh (needed by the
                # b+1 combine, even later) follow.
                for c in range(4):
                    expa_chunk(c)
                for h in range(2):
                    nc.scalar.activation(
                        tanh_t[:, h * 4 : (h + 1) * 4, :],
                        qb_all[:, h * 4 : (h + 1) * 4, :],
                        Act.Tanh,
                    )
                return expA, ekv, tanh_t

            def phase_main(bi, expA, ekv, tanh_t):
                out_t = out_p.tile([128, KT, D], BF16, tag="outst")
                for qi in range(KT):
                    mm = mpsum.tile([128, 512], F32, tag="mm")
                    for t in range(KT):
                        nc.tensor.matmul(
                            mm[:],
                            expA[:, t, qi * 128 : (qi + 1) * 128],
                            ekv[:, t, :],
                            start=(t == 0),
                            stop=(t == KT - 1),
                        )
                    r = eps_p.tile([128, D], F32, tag="r")
                    nc.vector.reciprocal_approx_fast(r[:], mm[:, D : 2 * D])
                    m = eps_p.tile([128, D], F32, tag="m")
                    nc.vector.tensor_mul(m[:], mm[:, 0:D], r[:])
                    a = eps_p.tile([128, D], F32, tag="a")
                    nc.gpsimd.tensor_add(a[:], m[:], bvh_t[:])
                    nc.vector.scalar_tensor_tensor(
                        out_t[:, qi, :], tanh_t[:, qi, :], 1.0, a[:],
                        Alu.add, Alu.mult,
                    )
                for h in range(2):
                    nc.gpsimd.dma_start(
                        out_d[bi, h * 512 : (h + 1) * 512, :].rearrange(
                            "(j p) e -> p j e", p=128
                        ),
                        out_t[:, h * 4 : (h + 1) * 4, :],
                    )

            # software pipeline: DMAs for b+1 are issued before main(b); the
            # PE runs main(b) (data already resident) and only then proj(b+1),
            # so it never head-of-line blocks on next-batch loads.
            staged = phase_proj(*phase_load(0, first=True))
            for bi in range(BPC):
                ld = phase_load(bi + 1) if bi + 1 < BPC else None
                phase_main(bi, *staged)
                staged = phase_proj(*ld) if ld is not None else None

    nc.compile()
    return nc


def prepare_in_maps(inputs: dict):
    query = np.asarray(inputs["query"], dtype=np.float32)
    key_ = np.asarray(inputs["key_"], dtype=np.float32)
    value = np.asarray(inputs["value"], dtype=np.float32)
    dist = np.asarray(inputs["dist"], dtype=np.float32)
    Wq = np.asarray(inputs["Wq"], dtype=np.float32)
    Wk = np.asarray(inputs["Wk"], dtype=np.float32)
    Wv = np.asarray(inputs["Wv"], dtype=np.float32)
    bq = np.asarray(inputs["bq"], dtype=np.float32)
    bv = np.asarray(inputs["bv"], dtype=np.float32)
    alpha_raw = np.asarray(inputs["alpha_raw"], dtype=np.float64)

    alpha = float(np.logaddexp(0.0, alpha_raw)) + 1e-6  # softplus + eps
    c_coef = float(alpha * np.log2(float(N)))

    distT = np.ascontiguousarray(dist.transpose(0, 2, 1))
    qT = np.ascontiguousarray(query.transpose(0, 2, 1))
    kT = np.ascontiguousarray(key_.transpose(0, 2, 1))
    vT = np.ascontiguousarray(value.transpose(0, 2, 1))
    WqT = np.ascontiguousarray(Wq.T)
    WkT = np.ascontiguousarray(Wk.T)
    WvT = np.ascontiguousarray(Wv.T) * 0.5  # fold sigmoid's 0.5
    # broadcast biases to 128 partitions; 0.5 folds sigmoid's scale
    bqh = np.ascontiguousarray(
        np.broadcast_to(0.5 * bq.reshape(1, D), (128, D)).astype(np.float32)
    )
    bvh = np.ascontiguousarray(
        np.broadcast_to(0.5 * bv.reshape(1, D), (128, D)).astype(np.float32)
    )

    in_maps = []
    for i in range(N_CORES):
        s = slice(i * BPC, (i + 1) * BPC)
        in_maps.append(
            {
                "distT": distT[s],
                "qT": qT[s],
                "kT": kT[s],
                "vT": vT[s],
                "WqT": WqT,
                "WkT": WkT,
                "WvT": WvT,
                "bqh": bqh,
                "bvh": bvh,
            }
        )
    return in_maps, c_coef


def run_sharded(inputs: dict, trace: bool = False):
    """Returns (full_output [32,1024,256] f32, BassKernelResults)."""
    in_maps, c_coef = prepare_in_maps(inputs)
    nc = build_graph(c_coef)
    res = run_bass_kernel_spmd(
        nc, in_maps, core_ids=list(range(N_CORES)), trace=trace
    )
    out = np.concatenate(
        [np.asarray(res.results[i]["out"]) for i in range(N_CORES)], axis=0
    ).astype(np.float32)
    return out, res


def kernel(**inputs) -> np.ndarray:
    out, _ = run_sharded(inputs, trace=False)
    return out
